# revision 6
# baseline (speedup 1.0000x reference)
"""Multi-head causal attention (nn_Attention_29583734734990) on 8 Trainium2 cores.

Sharding: core c -> batch b = c//2, head half hh = c%2 (8 of 16 heads, as 4
head-pairs). Each core computes its partial output sum_{h in its 8 heads}
softmax(QK^T/sqrt(d), causal) V W_o[h] for its batch; the host adds the two
half-head partials per batch.

Schedule: software-pipelined by issue order. The attention inner loop's PE
work (scores + PV) is matched 1:1 by ACT exp rows, so with a naive order the
PE stalls on every block waiting for exp (and de-ramps its clock). Instead,
PE-only work -- the next pair's Q/K/V projection chains, V transposes, and
the output projection -- is kept in a FIFO of filler closures and pumped
between the score and PV matmuls of each block; PV is additionally delayed
one block so exp latency is fully hidden. Fence markers in the FIFO force
any chain a given q-chunk depends on to be issued before its first score.

Dtypes: bf16 PE datapath with fp32 PSUM accumulation (same per-row matmul
rate as fp32r, but no N>=256 full-rate restriction, so causally-dead columns
are trimmed to the exact 128-col diagonal strip). resid is transposed as
fp32r (1.5 c/row) straight from the fp32 DMA and converted to bf16 in the
PSUM->SBUF copy.

PSUM (8 banks): 2x score tiles [128,1024] (4) + 2x z accum [65,512] (2) +
2x aux [128,512] (2) shared by transposes/projections/output chains.
"""
from collections import deque
from contextlib import ExitStack

import numpy as np

import concourse.bass as bass
import concourse.mybir as mybir
import concourse.tile as tile
from concourse.bass_utils import run_bass_kernel_spmd
from concourse.masks import make_identity

FP32 = mybir.dt.float32
FP32R = mybir.dt.float32r
BF16 = mybir.dt.bfloat16
EXP = mybir.ActivationFunctionType.Exp
COPY = mybir.ActivationFunctionType.Copy

B, S, M, D, H = 4, 2048, 1024, 64, 16
P = 128
NP = 4          # head pairs per core
MC = M // P     # 8  m chunks
KB = S // P     # 16 k blocks
QC = S // 512   # 4  q chunks


def _split_multiwait_instructions(nc):
    """This walrus build rejects instructions carrying >1 sem-wait ("Too many
    sync wait commands"). Move extra waits onto single-wait NoOps inserted just
    before on the same engine queue (identical semantics)."""
    ctr = 0
    for fn in nc.m.functions:
        for bb in fn.blocks:
            new = []
            for inst in list(bb.instructions):
                si = inst.sync_info
                if si is not None and len(si.on_wait) > 1:
                    waits = list(si.on_wait)
                    for w in waits[:-1]:
                        ctr += 1
                        new.append(
                            mybir.InstNoOp(
                                name=f"I-splitw-{ctr}",
                                engine=inst.engine,
                                bass_nofuse=True,
                                sync_info=mybir.SyncInfo(on_wait=[w], on_update=[]),
                            )
                        )
                    inst.sync_info = mybir.SyncInfo(
                        on_wait=[waits[-1]], on_update=list(si.on_update)
                    )
                new.append(inst)
            bb.instructions = new
    return ctr


def _body(tc, nc, resid_d, wq_d, wk_d, wv_d, wo_d, out_d):
    with ExitStack() as ctx:
        const = ctx.enter_context(tc.tile_pool(name="const", bufs=1))
        ident_f = const.tile([P, P], FP32, name="ident_f")
        make_identity(nc, ident_f[:])
        ident_r = const.tile([P, P], FP32R, name="ident_r")
        nc.vector.tensor_copy(ident_r[:], ident_f[:])
        big = ctx.enter_context(tc.tile_pool(name="big", bufs=4))
        residT = [
            big.tile([P, MC, 512], BF16, tag="residT", name=f"residT{g}")
            for g in range(4)
        ]

        qk_pool = ctx.enter_context(tc.tile_pool(name="qk", bufs=2))
        vt_pool = ctx.enter_context(tc.tile_pool(name="vt", bufs=2))
        vx_pool = ctx.enter_context(tc.tile_pool(name="vx", bufs=2))
        z_pool = ctx.enter_context(tc.tile_pool(name="zsb", bufs=NP))
        wo_pool = ctx.enter_context(tc.tile_pool(name="wop", bufs=NP))
        wf_pool = ctx.enter_context(tc.tile_pool(name="wf", bufs=2))
        wr_pool = ctx.enter_context(tc.tile_pool(name="wr", bufs=2))
        rs_pool = ctx.enter_context(tc.tile_pool(name="rs", bufs=8))
        pt_pool = ctx.enter_context(tc.tile_pool(name="pt", bufs=3))
        rc_pool = ctx.enter_context(tc.tile_pool(name="rc", bufs=2))
        zt_pool = ctx.enter_context(tc.tile_pool(name="ztm", bufs=2))
        ob_pool = ctx.enter_context(tc.tile_pool(name="osb", bufs=3))

        ps_st = ctx.enter_context(tc.tile_pool(name="ps_st", bufs=2, space="PSUM"))
        ps_z = ctx.enter_context(tc.tile_pool(name="ps_z", bufs=2, space="PSUM"))
        ps_aux = ctx.enter_context(tc.tile_pool(name="ps_aux", bufs=2, space="PSUM"))

        # ------------- filler FIFO with fence markers -------------
        fill = deque()          # items: closures or ("mark", key)
        done_marks = set()

        def _run_one():
            item = fill.popleft()
            if isinstance(item, tuple):
                done_marks.add(item[1])
            else:
                item()

        def pump(n):
            for _ in range(n):
                if not fill:
                    return
                _run_one()

        def flush_until(key):
            while key not in done_marks and fill:
                _run_one()

        def flush():
            while fill:
                _run_one()

        # ------------- weights -------------
        def stage_weights(p):
            """Issue HBM loads for pair p's weights (SP + Pool queues)."""
            stg = {}
            for name, w_d, q in (
                ("wq", wq_d, nc.sync),
                ("wk", wk_d, nc.gpsimd),
                ("wv", wv_d, nc.sync),
            ):
                t = wf_pool.tile([P, MC, 2, D], FP32, tag=f"s{name}", name=f"s{name}")
                for h in range(2):
                    q.dma_start(
                        t[:, :, h, :],
                        w_d[2 * p + h].rearrange("(mc pp) d -> pp mc d", pp=P),
                    )
                stg[name] = t
            t = wf_pool.tile([P, M], FP32, tag="swo", name="swo")
            nc.gpsimd.dma_start(
                t[:], wo_d[2 * p:2 * p + 2].rearrange("h d m -> (h d) m")
            )
            stg["wo"] = t
            return stg

        wo_handles = []

        def convert_weights(stg, wr_out):
            """fp32 staging -> bf16 (DVE). Appends wq/wk/wv to wr_out, wo to
            wo_handles."""
            for name in ("wq", "wk", "wv"):
                wr = wr_pool.tile([P, MC, 2, D], BF16, tag=f"r{name}", name=f"r{name}")
                nc.vector.tensor_copy(wr[:], stg[name][:])
                wr_out.append(wr)
            wo_r = wo_pool.tile([P, M], BF16, tag="wo", name="wo_r")
            nc.vector.tensor_copy(wo_r[:], stg["wo"][:])
            wo_handles.append(wo_r)

        # ------------- lazy chain builders (alloc PSUM at first run) -------
        def proj_chain(wr_list, wi, T, sj):
            """T[:, sj*512:(sj+1)*512] = W^T @ residT[sj]: 2x(4 matmuls)+copy."""
            hold = {}

            def step(k):
                def f():
                    if "ps" not in hold:
                        hold["ps"] = ps_aux.tile([P, 512], FP32, tag="aux", name="aux")
                    ps = hold["ps"]
                    if k < 2:
                        wr = wr_list[wi]
                        for mi in range(4 * k, 4 * k + 4):
                            nc.tensor.matmul(
                                ps[:],
                                wr[:, mi].rearrange("pp h d -> pp (h d)"),
                                residT[sj][:, mi, :],
                                start=(mi == 0),
                                stop=(mi == MC - 1),
                            )
                    else:
                        nc.vector.tensor_copy(
                            T[:, sj * 512:(sj + 1) * 512], ps[:]
                        )
                return f

            return [step(0), step(1), step(2)]

        def vx_chain(VT, vx, kg):
            """vx[:, 4kg:4kg+4, :, 0:D] = transpose of 4 VT column blocks."""
            hold = {}

            def step(k):
                def f():
                    if "ps" not in hold:
                        hold["ps"] = ps_aux.tile([P, 512], FP32, tag="aux", name="aux")
                    tp = hold["ps"]
                    if k == 0:
                        for kbi in range(4):
                            kb = kg * 4 + kbi
                            nc.tensor.transpose(
                                tp[:, kbi * P:(kbi + 1) * P].bitcast(FP32R),
                                VT[:, kb * P:(kb + 1) * P],
                                ident_r[:],
                            )
                    else:
                        nc.vector.tensor_copy(
                            vx[:, kg * 4:(kg + 1) * 4, :, 0:D],
                            tp[:].rearrange(
                                "pp (kbi h d) -> pp kbi h d", kbi=4, h=2
                            ),
                        )
                return f

            return [step(0), step(1)]

        def out_chain(z_sbs, qb, mj):
            """out[qb*128:, mj*512:] = sum_p Z_p^T.T @ Wo_p."""
            hold = {}

            def step(k):
                def f():
                    if "ps" not in hold:
                        hold["ps"] = ps_aux.tile([P, 512], FP32, tag="aux", name="aux")
                    po = hold["ps"]
                    if k == 0:
                        for p4 in range(NP):
                            nc.tensor.matmul(
                                po[:],
                                z_sbs[p4][:, qb * P:(qb + 1) * P],
                                wo_handles[p4][:, mj * 512:(mj + 1) * 512],
                                start=(p4 == 0),
                                stop=(p4 == NP - 1),
                            )
                    else:
                        ob = ob_pool.tile([P, 512], FP32, tag="o", name="ob")
                        nc.vector.tensor_copy(ob[:], po[:])
                        nc.sync.dma_start(
                            out_d[qb * P:(qb + 1) * P,
                                  mj * 512:(mj + 1) * 512],
                            ob[:],
                        )
                return f

            return [step(0), step(1)]

        def enqueue_pair_chains(p, wr_list, QT, KT, VT, vx, direct_sg0=False):
            """Queue pair p's 12 projection chains + 4 vx chains, with a fence
            marker after each s-group."""
            for sg in range(4):
                thunks = (proj_chain(wr_list, 0, QT, sg)
                          + proj_chain(wr_list, 1, KT, sg)
                          + proj_chain(wr_list, 2, VT, sg)
                          + vx_chain(VT, vx, sg))
                if direct_sg0 and sg == 0:
                    for t in thunks:
                        t()
                    done_marks.add((p, 0))
                else:
                    fill.extend(thunks)
                    fill.append(("mark", (p, sg)))

        # ------------- pair 0 weights + resid transpose pipeline -----------
        stg0 = stage_weights(0)
        wr0 = []
        convert_weights(stg0, wr0)

        QTs = [qk_pool.tile([P, S], BF16, tag="qt", name="QT0")]
        KTs = [qk_pool.tile([P, S], BF16, tag="kt", name="KT0")]
        VTs = [vt_pool.tile([P, S], FP32R, tag="vtt", name="VT0")]
        vxs = [vx_pool.tile([P, KB, 2, D + 1], BF16, tag="vx", name="vx0")]
        nc.vector.memset(vxs[0][:, :, :, D:D + 1], 1.0)

        def load_rs_group(sg):
            tiles = [rs_pool.tile([P, M], FP32, tag="rs", name="rs") for _ in range(4)]
            for sci in range(4):
                q = nc.sync if sci % 2 == 0 else nc.gpsimd
                q.dma_start(
                    tiles[sci][:],
                    resid_d[(sg * 4 + sci) * P:(sg * 4 + sci + 1) * P, :],
                )
            return tiles

        rs_cur = load_rs_group(0)
        for sg in range(4):
            rs_nxt = load_rs_group(sg + 1) if sg < 3 else None
            for mi in range(MC):
                tp = ps_aux.tile([P, 512], FP32, tag="aux", name="tp")
                for sci in range(4):
                    nc.tensor.transpose(
                        tp[:, sci * P:(sci + 1) * P],
                        rs_cur[sci][:, mi * P:(mi + 1) * P],
                        ident_f[:],
                    )
                # alternate DVE/ACT for the fp32->bf16 convert-copy
                if mi % 2 == 0:
                    nc.vector.tensor_copy(residT[sg][:, mi, :], tp[:])
                else:
                    nc.scalar.activation(residT[sg][:, mi, :], tp[:], COPY)
            rs_cur = rs_nxt
            if sg == 0:
                enqueue_pair_chains(0, wr0, QTs[0], KTs[0], VTs[0], vxs[0],
                                    direct_sg0=True)

        # ------------- attention -------------
        z_sbs = []

        def normalize(zps, z_sb, qj):
            zsl = slice(qj * 512, (qj + 1) * 512)
            for h in range(2):
                rcp = rc_pool.tile([D + 1, 512], FP32, tag="rc", name="rcp")
                nc.vector.reciprocal(rcp[D:D + 1, :], zps[h][D:D + 1, :])
                Rs = rc_pool.tile([D, 512], FP32, tag="rsb", name="Rs")
                nc.sync.dma_start(
                    Rs[:], rcp[D:D + 1, None, :].to_broadcast((1, D, 512))
                )
                if h == 0:
                    nc.vector.tensor_mul(z_sb[0:D, zsl], zps[h][0:D, :], Rs[:])
                else:
                    ztmp = zt_pool.tile([D, 512], BF16, tag="zt", name="ztmp")
                    nc.vector.tensor_mul(ztmp[:], zps[h][0:D, :], Rs[:])
                    nc.sync.dma_start(z_sb[64:128, zsl], ztmp[:])

        def attention(p, QT, KT, vx, z_sb, qj_hooks):
            pending = None
            for qj in range(QC):
                flush_until((p, qj))
                hook = qj_hooks.get(qj)
                if hook:
                    hook()
                nkb = 4 * qj + 4
                zps = None
                for kb in range(nkb):
                    m = kb - 4 * qj
                    c0 = 0 if m < 1 else P * m
                    st = ps_st.tile([P, 1024], FP32, tag="st", name="st")
                    if kb == 0:
                        zps = [
                            ps_z.tile([D + 1, 512], FP32, tag="z", name=f"z{h}")
                            for h in range(2)
                        ]
                    for h in range(2):
                        nc.tensor.matmul(
                            st[:, h * 512 + c0:(h + 1) * 512],
                            KT[h * D:(h + 1) * D, kb * P:(kb + 1) * P],
                            QT[h * D:(h + 1) * D,
                               qj * 512 + c0:(qj + 1) * 512],
                            start=True,
                            stop=True,
                        )
                    pt = pt_pool.tile([P, 1024], BF16, tag="pt", name="pt")
                    if c0 > 0:
                        st3 = st[:].rearrange("pp (h c) -> pp h c", h=2)
                        pt3 = pt[:].rearrange("pp (h c) -> pp h c", h=2)
                        nc.scalar.activation(
                            pt3[:, :, c0:512], st3[:, :, c0:512], EXP,
                            scale=0.125,
                        )
                    else:
                        nc.scalar.activation(pt[:], st[:], EXP, scale=0.125)
                    if m >= 0:
                        # zero the upper-triangle of the 128-col diag strip
                        for h in range(2):
                            nc.gpsimd.affine_select(
                                out=pt[:, h * 512 + c0:h * 512 + c0 + P],
                                in_=pt[:, h * 512 + c0:h * 512 + c0 + P],
                                compare_op=mybir.AluOpType.is_ge,
                                fill=0.0,
                                base=0,
                                pattern=[[1, P]],
                                channel_multiplier=-1,
                            )
                    pump(2)
                    if pending is not None:
                        pending()

                    def mk_pv(zz, ptt, cc0, kkb, last):
                        def f():
                            for h in range(2):
                                nc.tensor.matmul(
                                    zz[h][:, cc0:512],
                                    vx[:, kkb, h, :],
                                    ptt[:, h * 512 + cc0:(h + 1) * 512],
                                    start=(kkb == 0),
                                    stop=last,
                                )
                        return f

                    pending = mk_pv(zps, pt, c0, kb, kb == nkb - 1)
                    if kb == nkb - 1:
                        # qj's last PV can't be delayed into the next block;
                        # extra filler covers the exp latency instead
                        pump(2)
                        pending()
                        pending = None
                        normalize(zps, z_sb, qj)

        for p in range(NP):
            z_sb = z_pool.tile([P, S], BF16, tag="z", name=f"zsb{p}")
            z_sbs.append(z_sb)

            if p < NP - 1:
                # queue next pair's weights + projections as filler
                pn = p + 1
                stg = stage_weights(pn)
                QTs.append(qk_pool.tile([P, S], BF16, tag="qt", name=f"QT{pn}"))
                KTs.append(qk_pool.tile([P, S], BF16, tag="kt", name=f"KT{pn}"))
                VTs.append(vt_pool.tile([P, S], FP32R, tag="vtt", name=f"VT{pn}"))
                vxs.append(vx_pool.tile([P, KB, 2, D + 1], BF16, tag="vx",
                                        name=f"vx{pn}"))
                wr_n = []

                def cvt(stg=stg, wr_n=wr_n, vxn=vxs[pn]):
                    convert_weights(stg, wr_n)
                    nc.vector.memset(vxn[:, :, :, D:D + 1], 1.0)

                fill.append(cvt)
                enqueue_pair_chains(pn, wr_n, QTs[pn], KTs[pn], VTs[pn],
                                    vxs[pn])

            qj_hooks = {}
            if p == NP - 1:
                # filler for the last pair: output projection of finished qj
                def mk_out_hook(qj_done):
                    def hk():
                        for qb in range(qj_done * 4, qj_done * 4 + 4):
                            for mj in range(2):
                                fill.extend(out_chain(z_sbs, qb, mj))
                    return hk

                for qj in range(1, QC):
                    qj_hooks[qj] = mk_out_hook(qj - 1)

            attention(p, QTs[p], KTs[p], vxs[p], z_sb, qj_hooks)

        # ------------- tail: last q-chunk's output projection -------------
        flush()
        for qb in range(12, KB):
            for mj in range(2):
                for t in out_chain(z_sbs, qb, mj):
                    t()


_NC_CACHE = None


def _build_nc(split_waits=True):
    global _NC_CACHE
    if _NC_CACHE is not None and split_waits:
        return _NC_CACHE
    nc = bass.Bass("TRN2", target_bir_lowering=False, debug=False, num_devices=8)
    resid_d = nc.dram_tensor("resid", [S, M], FP32, kind="ExternalInput").ap()
    wq_d = nc.dram_tensor("wq", [H // 2, M, D], FP32, kind="ExternalInput").ap()
    wk_d = nc.dram_tensor("wk", [H // 2, M, D], FP32, kind="ExternalInput").ap()
    wv_d = nc.dram_tensor("wv", [H // 2, M, D], FP32, kind="ExternalInput").ap()
    wo_d = nc.dram_tensor("wo", [H // 2, D, M], FP32, kind="ExternalInput").ap()
    out_d = nc.dram_tensor("out", [S, M], FP32, kind="ExternalOutput").ap()
    with tile.TileContext(nc) as tc:
        _body(tc, nc, resid_d, wq_d, wk_d, wv_d, wo_d, out_d)
    if split_waits:
        _split_multiwait_instructions(nc)
        _NC_CACHE = nc
    return nc


def run(resid, w_q, w_k, w_v, w_o, **spmd_kwargs):
    """Build + run on 8 cores; returns (full output [4,2048,1024], BassKernelResults)."""
    resid = np.asarray(resid, dtype=np.float32)
    w_q = np.asarray(w_q, dtype=np.float32)
    w_k = np.asarray(w_k, dtype=np.float32)
    w_v = np.asarray(w_v, dtype=np.float32)
    w_o = np.asarray(w_o, dtype=np.float32)

    nc = _build_nc()
    in_maps = []
    for c in range(8):
        b, hh = c // 2, c % 2
        hs = slice(8 * hh, 8 * hh + 8)
        in_maps.append(
            {
                "resid": np.ascontiguousarray(resid[b]),
                "wq": np.ascontiguousarray(w_q[hs]),
                "wk": np.ascontiguousarray(w_k[hs]),
                "wv": np.ascontiguousarray(w_v[hs]),
                "wo": np.ascontiguousarray(w_o[hs]),
            }
        )
    res = run_bass_kernel_spmd(nc, in_maps, core_ids=list(range(8)), **spmd_kwargs)
    outs = [r["out"] for r in res.results]
    full = np.stack([outs[2 * b] + outs[2 * b + 1] for b in range(B)])
    return full.astype(np.float32), res


def kernel(resid, w_q, w_k, w_v, w_o):
    full, _ = run(resid, w_q, w_k, w_v, w_o)
    return full


# revision 7
# speedup vs baseline: 1.1077x; 1.1077x over previous
"""Multi-head causal attention (nn_Attention_29583734734990) on 8 Trainium2 cores.

Sharding: core c -> batch b = c//2, head half hh = c%2 (8 of 16 heads, as 4
head-pairs). Each core computes its partial output sum_{h in its 8 heads}
softmax(QK^T/sqrt(d), causal) V W_o[h] for its batch; the host adds the two
half-head partials per batch.

Schedule: software-pipelined by issue order. The attention inner loop's PE
work (scores + PV) is matched 1:1 by ACT exp rows, so with a naive order the
PE stalls on every block waiting for exp (and de-ramps its clock). Instead,
PE-only work -- the next pair's Q/K/V projection chains, V transposes, and
the output projection -- is kept in a FIFO of filler closures and pumped
between the score and PV matmuls of each block; PV is additionally delayed
one block so exp latency is fully hidden. Fence markers in the FIFO force
any chain a given q-chunk depends on to be issued before its first score.

Dtypes: bf16 PE datapath with fp32 PSUM accumulation (same per-row matmul
rate as fp32r, but no N>=256 full-rate restriction, so causally-dead columns
are trimmed to the exact 128-col diagonal strip). resid is transposed as
fp32r (1.5 c/row) straight from the fp32 DMA and converted to bf16 in the
PSUM->SBUF copy.

PSUM (8 banks): 2x score tiles [128,1024] (4) + 2x z accum [65,512] (2) +
2x aux [128,512] (2) shared by transposes/projections/output chains.
"""
from collections import deque
from contextlib import ExitStack

import numpy as np

import concourse.bass as bass
import concourse.mybir as mybir
import concourse.tile as tile
from concourse.bass_utils import run_bass_kernel_spmd
from concourse.masks import make_identity

FP32 = mybir.dt.float32
FP32R = mybir.dt.float32r
BF16 = mybir.dt.bfloat16
EXP = mybir.ActivationFunctionType.Exp
COPY = mybir.ActivationFunctionType.Copy

B, S, M, D, H = 4, 2048, 1024, 64, 16
P = 128
NP = 4          # head pairs per core
MC = M // P     # 8  m chunks
KB = S // P     # 16 k blocks
QC = S // 512   # 4  q chunks


def _split_multiwait_instructions(nc):
    """This walrus build rejects instructions carrying >1 sem-wait ("Too many
    sync wait commands"). Move extra waits onto single-wait NoOps inserted just
    before on the same engine queue (identical semantics)."""
    ctr = 0
    for fn in nc.m.functions:
        for bb in fn.blocks:
            new = []
            for inst in list(bb.instructions):
                si = inst.sync_info
                if si is not None and len(si.on_wait) > 1:
                    waits = list(si.on_wait)
                    for w in waits[:-1]:
                        ctr += 1
                        new.append(
                            mybir.InstNoOp(
                                name=f"I-splitw-{ctr}",
                                engine=inst.engine,
                                bass_nofuse=True,
                                sync_info=mybir.SyncInfo(on_wait=[w], on_update=[]),
                            )
                        )
                    inst.sync_info = mybir.SyncInfo(
                        on_wait=[waits[-1]], on_update=list(si.on_update)
                    )
                new.append(inst)
            bb.instructions = new
    return ctr


def _body(tc, nc, resid_d, wq_d, wk_d, wv_d, wo_d, out_d):
    with ExitStack() as ctx:
        const = ctx.enter_context(tc.tile_pool(name="const", bufs=1))
        ident_f = const.tile([P, P], FP32, name="ident_f")
        make_identity(nc, ident_f[:])
        ident_r = const.tile([P, P], FP32R, name="ident_r")
        nc.vector.tensor_copy(ident_r[:], ident_f[:])
        big = ctx.enter_context(tc.tile_pool(name="big", bufs=4))
        residT = [
            big.tile([P, MC, 512], BF16, tag="residT", name=f"residT{g}")
            for g in range(4)
        ]

        qk_pool = ctx.enter_context(tc.tile_pool(name="qk", bufs=2))
        vt_pool = ctx.enter_context(tc.tile_pool(name="vt", bufs=2))
        vx_pool = ctx.enter_context(tc.tile_pool(name="vx", bufs=2))
        z_pool = ctx.enter_context(tc.tile_pool(name="zsb", bufs=NP))
        wo_pool = ctx.enter_context(tc.tile_pool(name="wop", bufs=NP))
        wf_pool = ctx.enter_context(tc.tile_pool(name="wf", bufs=2))
        wr_pool = ctx.enter_context(tc.tile_pool(name="wr", bufs=2))
        rs_pool = ctx.enter_context(tc.tile_pool(name="rs", bufs=8))
        pt_pool = ctx.enter_context(tc.tile_pool(name="pt", bufs=3))
        rc_pool = ctx.enter_context(tc.tile_pool(name="rc", bufs=2))
        zt_pool = ctx.enter_context(tc.tile_pool(name="ztm", bufs=2))
        ob_pool = ctx.enter_context(tc.tile_pool(name="osb", bufs=3))

        ps_st = ctx.enter_context(tc.tile_pool(name="ps_st", bufs=2, space="PSUM"))
        ps_z = ctx.enter_context(tc.tile_pool(name="ps_z", bufs=2, space="PSUM"))
        ps_aux = ctx.enter_context(tc.tile_pool(name="ps_aux", bufs=2, space="PSUM"))

        # ------------- filler FIFO with fence markers -------------
        fill = deque()          # items: closures or ("mark", key)
        done_marks = set()

        def _run_one():
            item = fill.popleft()
            if isinstance(item, tuple):
                done_marks.add(item[1])
            else:
                item()

        def pump(n):
            for _ in range(n):
                if not fill:
                    return
                _run_one()

        def flush_until(key):
            while key not in done_marks and fill:
                _run_one()

        def flush():
            while fill:
                _run_one()

        # ------------- weights -------------
        def stage_weights(p):
            """Issue HBM loads for pair p's weights (SP + Pool queues)."""
            stg = {}
            for name, w_d, q in (
                ("wq", wq_d, nc.sync),
                ("wk", wk_d, nc.gpsimd),
                ("wv", wv_d, nc.sync),
            ):
                t = wf_pool.tile([P, MC, 2, D], FP32, tag=f"s{name}", name=f"s{name}")
                for h in range(2):
                    q.dma_start(
                        t[:, :, h, :],
                        w_d[2 * p + h].rearrange("(mc pp) d -> pp mc d", pp=P),
                    )
                stg[name] = t
            t = wf_pool.tile([P, M], FP32, tag="swo", name="swo")
            nc.gpsimd.dma_start(
                t[:], wo_d[2 * p:2 * p + 2].rearrange("h d m -> (h d) m")
            )
            stg["wo"] = t
            return stg

        def load_rs_group(sg):
            tiles = [rs_pool.tile([P, M], FP32, tag="rs", name="rs") for _ in range(4)]
            for sci in range(4):
                q = nc.sync if sci % 2 == 0 else nc.gpsimd
                q.dma_start(
                    tiles[sci][:],
                    resid_d[(sg * 4 + sci) * P:(sg * 4 + sci + 1) * P, :],
                )
            return tiles

        wo_handles = []

        def convert_weights(stg, wr_out):
            """fp32 staging -> bf16 (DVE). Appends wq/wk/wv to wr_out, wo to
            wo_handles."""
            for name in ("wq", "wk", "wv"):
                wr = wr_pool.tile([P, MC, 2, D], BF16, tag=f"r{name}", name=f"r{name}")
                nc.vector.tensor_copy(wr[:], stg[name][:])
                wr_out.append(wr)
            wo_r = wo_pool.tile([P, M], BF16, tag="wo", name="wo_r")
            nc.vector.tensor_copy(wo_r[:], stg["wo"][:])
            wo_handles.append(wo_r)

        # ------------- lazy chain builders (alloc PSUM at first run) -------
        def proj_chain(wr_list, wi, T, sj):
            """T[:, sj*512:(sj+1)*512] = W^T @ residT[sj]: 2x(4 matmuls)+copy."""
            hold = {}

            def step(k):
                def f():
                    if "ps" not in hold:
                        hold["ps"] = ps_aux.tile([P, 512], FP32, tag="aux", name="aux")
                    ps = hold["ps"]
                    if k < 4:
                        wr = wr_list[wi]
                        for mi in range(2 * k, 2 * k + 2):
                            nc.tensor.matmul(
                                ps[:],
                                wr[:, mi].rearrange("pp h d -> pp (h d)"),
                                residT[sj][:, mi, :],
                                start=(mi == 0),
                                stop=(mi == MC - 1),
                            )
                    else:
                        nc.vector.tensor_copy(
                            T[:, sj * 512:(sj + 1) * 512], ps[:]
                        )
                return f

            return [step(0), step(1), step(2), step(3), step(4)]

        def vx_chain(VT, vx, kg):
            """vx[:, 4kg:4kg+4, :, 0:D] = transpose of 4 VT column blocks."""
            hold = {}

            def step(k):
                def f():
                    if "ps" not in hold:
                        hold["ps"] = ps_aux.tile([P, 512], FP32, tag="aux", name="aux")
                    tp = hold["ps"]
                    if k == 0:
                        for kbi in range(4):
                            kb = kg * 4 + kbi
                            nc.tensor.transpose(
                                tp[:, kbi * P:(kbi + 1) * P].bitcast(FP32R),
                                VT[:, kb * P:(kb + 1) * P],
                                ident_r[:],
                            )
                    else:
                        nc.vector.tensor_copy(
                            vx[:, kg * 4:(kg + 1) * 4, :, 0:D],
                            tp[:].rearrange(
                                "pp (kbi h d) -> pp kbi h d", kbi=4, h=2
                            ),
                        )
                return f

            return [step(0), step(1)]

        def out_chain(z_sbs, qb, mj):
            """out[qb*128:, mj*512:] = sum_p Z_p^T.T @ Wo_p."""
            hold = {}

            def step(k):
                def f():
                    if "ps" not in hold:
                        hold["ps"] = ps_aux.tile([P, 512], FP32, tag="aux", name="aux")
                    po = hold["ps"]
                    if k < 2:
                        for p4 in range(2 * k, 2 * k + 2):
                            nc.tensor.matmul(
                                po[:],
                                z_sbs[p4][:, qb * P:(qb + 1) * P],
                                wo_handles[p4][:, mj * 512:(mj + 1) * 512],
                                start=(p4 == 0),
                                stop=(p4 == NP - 1),
                            )
                    else:
                        ob = ob_pool.tile([P, 512], FP32, tag="o", name="ob")
                        nc.vector.tensor_copy(ob[:], po[:])
                        nc.sync.dma_start(
                            out_d[qb * P:(qb + 1) * P,
                                  mj * 512:(mj + 1) * 512],
                            ob[:],
                        )
                return f

            return [step(0), step(1), step(2)]

        def enqueue_pair_chains(p, wr_list, QT, KT, VT, vx, direct_sg0=False):
            """Queue pair p's 12 projection chains + 4 vx chains, with a fence
            marker after each s-group."""
            for sg in range(4):
                thunks = (proj_chain(wr_list, 0, QT, sg)
                          + proj_chain(wr_list, 1, KT, sg)
                          + proj_chain(wr_list, 2, VT, sg)
                          + vx_chain(VT, vx, sg))
                if direct_sg0 and sg == 0:
                    for t in thunks:
                        t()
                    done_marks.add((p, 0))
                else:
                    fill.extend(thunks)
                    fill.append(("mark", (p, sg)))

        # ------------- pair 0 weights + resid transpose pipeline -----------
        rs_first = load_rs_group(0)
        stg0 = stage_weights(0)
        wr0 = []
        convert_weights(stg0, wr0)

        QTs = [qk_pool.tile([P, S], BF16, tag="qt", name="QT0")]
        KTs = [qk_pool.tile([P, S], BF16, tag="kt", name="KT0")]
        VTs = [vt_pool.tile([P, S], FP32R, tag="vtt", name="VT0")]
        vxs = [vx_pool.tile([P, KB, 2, D + 1], BF16, tag="vx", name="vx0")]
        nc.vector.memset(vxs[0][:, :, :, D:D + 1], 1.0)

        rs_cur = rs_first
        for sg in range(4):
            rs_nxt = load_rs_group(sg + 1) if sg < 3 else None
            for mi in range(MC):
                tp = ps_aux.tile([P, 512], FP32, tag="aux", name="tp")
                for sci in range(4):
                    nc.tensor.transpose(
                        tp[:, sci * P:(sci + 1) * P],
                        rs_cur[sci][:, mi * P:(mi + 1) * P],
                        ident_f[:],
                    )
                # alternate DVE/ACT for the fp32->bf16 convert-copy
                if mi % 2 == 0:
                    nc.vector.tensor_copy(residT[sg][:, mi, :], tp[:])
                else:
                    nc.scalar.activation(residT[sg][:, mi, :], tp[:], COPY)
            rs_cur = rs_nxt
            if sg == 0:
                enqueue_pair_chains(0, wr0, QTs[0], KTs[0], VTs[0], vxs[0],
                                    direct_sg0=True)

        # ------------- attention -------------
        z_sbs = []

        def normalize(zps, z_sb, qj):
            zsl = slice(qj * 512, (qj + 1) * 512)
            for h in range(2):
                rcp = rc_pool.tile([D + 1, 512], FP32, tag="rc", name="rcp")
                nc.vector.reciprocal(rcp[D:D + 1, :], zps[h][D:D + 1, :])
                Rs = rc_pool.tile([D, 512], FP32, tag="rsb", name="Rs")
                nc.sync.dma_start(
                    Rs[:], rcp[D:D + 1, None, :].to_broadcast((1, D, 512))
                )
                if h == 0:
                    nc.vector.tensor_mul(z_sb[0:D, zsl], zps[h][0:D, :], Rs[:])
                else:
                    ztmp = zt_pool.tile([D, 512], BF16, tag="zt", name="ztmp")
                    nc.vector.tensor_mul(ztmp[:], zps[h][0:D, :], Rs[:])
                    nc.sync.dma_start(z_sb[64:128, zsl], ztmp[:])

        def attention(p, QT, KT, vx, z_sb, qj_hooks, last_pair=False):
            pending = None
            for qj in range(QC):
                flush_until((p, qj))
                hook = qj_hooks.get(qj)
                if hook:
                    hook()
                nkb = 4 * qj + 4
                zps = None
                for kb in range(nkb):
                    m = kb - 4 * qj
                    c0 = 0 if m < 1 else P * m
                    st = ps_st.tile([P, 1024], FP32, tag="st", name="st")
                    if kb == 0:
                        zps = [
                            ps_z.tile([D + 1, 512], FP32, tag="z", name=f"z{h}")
                            for h in range(2)
                        ]
                    for h in range(2):
                        nc.tensor.matmul(
                            st[:, h * 512 + c0:(h + 1) * 512],
                            KT[h * D:(h + 1) * D, kb * P:(kb + 1) * P],
                            QT[h * D:(h + 1) * D,
                               qj * 512 + c0:(qj + 1) * 512],
                            start=True,
                            stop=True,
                        )
                    pt = pt_pool.tile([P, 1024], BF16, tag="pt", name="pt")
                    if c0 > 0:
                        st3 = st[:].rearrange("pp (h c) -> pp h c", h=2)
                        pt3 = pt[:].rearrange("pp (h c) -> pp h c", h=2)
                        nc.scalar.activation(
                            pt3[:, :, c0:512], st3[:, :, c0:512], EXP,
                            scale=0.125,
                        )
                    else:
                        nc.scalar.activation(pt[:], st[:], EXP, scale=0.125)
                    if m >= 0:
                        # zero the upper-triangle of the 128-col diag strip
                        for h in range(2):
                            nc.gpsimd.affine_select(
                                out=pt[:, h * 512 + c0:h * 512 + c0 + P],
                                in_=pt[:, h * 512 + c0:h * 512 + c0 + P],
                                compare_op=mybir.AluOpType.is_ge,
                                fill=0.0,
                                base=0,
                                pattern=[[1, P]],
                                channel_multiplier=-1,
                            )
                    # pair 3's out-chain filler must not run in the first
                    # blocks of a qj: it would stall on the normalize of the
                    # qj that just finished
                    if not (last_pair and qj > 0 and kb < 3):
                        pump(1)
                    if pending is not None:
                        pending()

                    def mk_pv(zz, ptt, cc0, kkb, last):
                        def f():
                            for h in range(2):
                                nc.tensor.matmul(
                                    zz[h][:, cc0:512],
                                    vx[:, kkb, h, :],
                                    ptt[:, h * 512 + cc0:(h + 1) * 512],
                                    start=(kkb == 0),
                                    stop=last,
                                )
                        return f

                    pending = mk_pv(zps, pt, c0, kb, kb == nkb - 1)
                    if kb == nkb - 1:
                        # qj's last PV can't be delayed into the next block;
                        # extra filler covers the exp latency instead
                        pump(1)
                        pending()
                        pending = None
                        normalize(zps, z_sb, qj)
                        pump(2)

        for p in range(NP):
            z_sb = z_pool.tile([P, S], BF16, tag="z", name=f"zsb{p}")
            z_sbs.append(z_sb)

            if p < NP - 1:
                # queue next pair's weights + projections as filler
                pn = p + 1
                stg = stage_weights(pn)
                QTs.append(qk_pool.tile([P, S], BF16, tag="qt", name=f"QT{pn}"))
                KTs.append(qk_pool.tile([P, S], BF16, tag="kt", name=f"KT{pn}"))
                VTs.append(vt_pool.tile([P, S], FP32R, tag="vtt", name=f"VT{pn}"))
                vxs.append(vx_pool.tile([P, KB, 2, D + 1], BF16, tag="vx",
                                        name=f"vx{pn}"))
                wr_n = []

                def cvt(stg=stg, wr_n=wr_n, vxn=vxs[pn]):
                    convert_weights(stg, wr_n)
                    nc.vector.memset(vxn[:, :, :, D:D + 1], 1.0)

                fill.append(cvt)
                enqueue_pair_chains(pn, wr_n, QTs[pn], KTs[pn], VTs[pn],
                                    vxs[pn])

            qj_hooks = {}
            if p == NP - 1:
                # filler for the last pair: output projection of finished qj
                def mk_out_hook(qj_done):
                    def hk():
                        for qb in range(qj_done * 4, qj_done * 4 + 4):
                            for mj in range(2):
                                fill.extend(out_chain(z_sbs, qb, mj))
                    return hk

                for qj in range(1, QC):
                    qj_hooks[qj] = mk_out_hook(qj - 1)

            attention(p, QTs[p], KTs[p], vxs[p], z_sb, qj_hooks,
                      last_pair=(p == NP - 1))

        # ------------- tail: last q-chunk's output projection -------------
        flush()
        for qb in range(12, KB):
            for mj in range(2):
                for t in out_chain(z_sbs, qb, mj):
                    t()


_NC_CACHE = None


def _build_nc(split_waits=True):
    global _NC_CACHE
    if _NC_CACHE is not None and split_waits:
        return _NC_CACHE
    nc = bass.Bass("TRN2", target_bir_lowering=False, debug=False, num_devices=8)
    resid_d = nc.dram_tensor("resid", [S, M], FP32, kind="ExternalInput").ap()
    wq_d = nc.dram_tensor("wq", [H // 2, M, D], FP32, kind="ExternalInput").ap()
    wk_d = nc.dram_tensor("wk", [H // 2, M, D], FP32, kind="ExternalInput").ap()
    wv_d = nc.dram_tensor("wv", [H // 2, M, D], FP32, kind="ExternalInput").ap()
    wo_d = nc.dram_tensor("wo", [H // 2, D, M], FP32, kind="ExternalInput").ap()
    out_d = nc.dram_tensor("out", [S, M], FP32, kind="ExternalOutput").ap()
    with tile.TileContext(nc) as tc:
        _body(tc, nc, resid_d, wq_d, wk_d, wv_d, wo_d, out_d)
    if split_waits:
        _split_multiwait_instructions(nc)
        _NC_CACHE = nc
    return nc


def run(resid, w_q, w_k, w_v, w_o, **spmd_kwargs):
    """Build + run on 8 cores; returns (full output [4,2048,1024], BassKernelResults)."""
    resid = np.asarray(resid, dtype=np.float32)
    w_q = np.asarray(w_q, dtype=np.float32)
    w_k = np.asarray(w_k, dtype=np.float32)
    w_v = np.asarray(w_v, dtype=np.float32)
    w_o = np.asarray(w_o, dtype=np.float32)

    nc = _build_nc()
    in_maps = []
    for c in range(8):
        b, hh = c // 2, c % 2
        hs = slice(8 * hh, 8 * hh + 8)
        in_maps.append(
            {
                "resid": np.ascontiguousarray(resid[b]),
                "wq": np.ascontiguousarray(w_q[hs]),
                "wk": np.ascontiguousarray(w_k[hs]),
                "wv": np.ascontiguousarray(w_v[hs]),
                "wo": np.ascontiguousarray(w_o[hs]),
            }
        )
    res = run_bass_kernel_spmd(nc, in_maps, core_ids=list(range(8)), **spmd_kwargs)
    outs = [r["out"] for r in res.results]
    full = np.stack([outs[2 * b] + outs[2 * b + 1] for b in range(B)])
    return full.astype(np.float32), res


def kernel(resid, w_q, w_k, w_v, w_o):
    full, _ = run(resid, w_q, w_k, w_v, w_o)
    return full


# revision 10
# speedup vs baseline: 1.1141x; 1.0058x over previous
"""Multi-head causal attention (nn_Attention_29583734734990) on 8 Trainium2 cores.

Sharding: core c -> batch b = c//2, head half hh = c%2 (8 of 16 heads, as 4
head-pairs). Each core computes its partial output sum_{h in its 8 heads}
softmax(QK^T/sqrt(d), causal) V W_o[h] for its batch; the host adds the two
half-head partials per batch.

Schedule: software-pipelined by issue order. The attention inner loop's PE
work (scores + PV) is matched 1:1 by ACT exp rows, so with a naive order the
PE stalls on every block waiting for exp (and de-ramps its clock). Instead,
PE-only work -- the next pair's Q/K/V projection chains, V transposes, and
the output projection -- is kept in a FIFO of filler closures and pumped
between the score and PV matmuls of each block; PV is additionally delayed
one block so exp latency is fully hidden. Fence markers in the FIFO force
any chain a given q-chunk depends on to be issued before its first score.

Dtypes: bf16 PE datapath with fp32 PSUM accumulation (same per-row matmul
rate as fp32r, but no N>=256 full-rate restriction, so causally-dead columns
are trimmed to the exact 128-col diagonal strip). resid is transposed as
fp32r (1.5 c/row) straight from the fp32 DMA and converted to bf16 in the
PSUM->SBUF copy.

PSUM (8 banks): 2x score tiles [128,1024] (4) + 2x z accum [65,512] (2) +
2x aux [128,512] (2) shared by transposes/projections/output chains.
"""
from collections import deque
from contextlib import ExitStack

import numpy as np

import concourse.bass as bass
import concourse.mybir as mybir
import concourse.tile as tile
from concourse.bass_utils import run_bass_kernel_spmd
from concourse.masks import make_identity

FP32 = mybir.dt.float32
FP32R = mybir.dt.float32r
BF16 = mybir.dt.bfloat16
EXP = mybir.ActivationFunctionType.Exp
COPY = mybir.ActivationFunctionType.Copy

B, S, M, D, H = 4, 2048, 1024, 64, 16
P = 128
NP = 4          # head pairs per core
MC = M // P     # 8  m chunks
KB = S // P     # 16 k blocks
QC = S // 512   # 4  q chunks


def _split_multiwait_instructions(nc):
    """This walrus build rejects instructions carrying >1 sem-wait ("Too many
    sync wait commands"). Move extra waits onto single-wait NoOps inserted just
    before on the same engine queue (identical semantics)."""
    ctr = 0
    for fn in nc.m.functions:
        for bb in fn.blocks:
            new = []
            for inst in list(bb.instructions):
                si = inst.sync_info
                if si is not None and len(si.on_wait) > 1:
                    waits = list(si.on_wait)
                    for w in waits[:-1]:
                        ctr += 1
                        new.append(
                            mybir.InstNoOp(
                                name=f"I-splitw-{ctr}",
                                engine=inst.engine,
                                bass_nofuse=True,
                                sync_info=mybir.SyncInfo(on_wait=[w], on_update=[]),
                            )
                        )
                    inst.sync_info = mybir.SyncInfo(
                        on_wait=[waits[-1]], on_update=list(si.on_update)
                    )
                new.append(inst)
            bb.instructions = new
    return ctr


def _body(tc, nc, resid_d, wq_d, wk_d, wv_d, wo_d, out_d):
    with ExitStack() as ctx:
        const = ctx.enter_context(tc.tile_pool(name="const", bufs=1))
        ident_f = const.tile([P, P], FP32, name="ident_f")
        make_identity(nc, ident_f[:])
        ident_r = const.tile([P, P], FP32R, name="ident_r")
        nc.vector.tensor_copy(ident_r[:], ident_f[:])
        big = ctx.enter_context(tc.tile_pool(name="big", bufs=4))
        residT = [
            big.tile([P, MC, 512], BF16, tag="residT", name=f"residT{g}")
            for g in range(4)
        ]

        qk_pool = ctx.enter_context(tc.tile_pool(name="qk", bufs=2))
        vt_pool = ctx.enter_context(tc.tile_pool(name="vt", bufs=1))
        vx_pool = ctx.enter_context(tc.tile_pool(name="vx", bufs=2))
        z_pool = ctx.enter_context(tc.tile_pool(name="zsb", bufs=NP))
        wo_pool = ctx.enter_context(tc.tile_pool(name="wop", bufs=NP))
        wf_pool = ctx.enter_context(tc.tile_pool(name="wf", bufs=1))
        wr_pool = ctx.enter_context(tc.tile_pool(name="wr", bufs=2))
        rs_pool = ctx.enter_context(tc.tile_pool(name="rs", bufs=16))
        pt_pool = ctx.enter_context(tc.tile_pool(name="pt", bufs=3))
        rc_pool = ctx.enter_context(tc.tile_pool(name="rc", bufs=2))
        zt_pool = ctx.enter_context(tc.tile_pool(name="ztm", bufs=2))
        ob_pool = ctx.enter_context(tc.tile_pool(name="osb", bufs=3))

        ps_st = ctx.enter_context(tc.tile_pool(name="ps_st", bufs=2, space="PSUM"))
        ps_z = ctx.enter_context(tc.tile_pool(name="ps_z", bufs=2, space="PSUM"))
        ps_aux = ctx.enter_context(tc.tile_pool(name="ps_aux", bufs=2, space="PSUM"))

        # ------------- filler FIFO with fence markers -------------
        fill = deque()          # items: closures or ("mark", key)
        done_marks = set()

        def _run_one():
            item = fill.popleft()
            if isinstance(item, tuple):
                done_marks.add(item[1])
            else:
                item()

        def pump(n):
            for _ in range(n):
                if not fill:
                    return
                _run_one()

        def flush_until(key):
            while key not in done_marks and fill:
                _run_one()

        def flush():
            while fill:
                _run_one()

        # ------------- weights -------------
        def stage_weights(p):
            """Issue HBM loads for pair p's weights (SP + Pool queues)."""
            stg = {}
            for name, w_d, q in (
                ("wq", wq_d, nc.sync),
                ("wk", wk_d, nc.gpsimd),
                ("wv", wv_d, nc.sync),
            ):
                t = wf_pool.tile([P, MC, 2, D], FP32, tag=f"s{name}", name=f"s{name}")
                for h in range(2):
                    q.dma_start(
                        t[:, :, h, :],
                        w_d[2 * p + h].rearrange("(mc pp) d -> pp mc d", pp=P),
                    )
                stg[name] = t
            t = wf_pool.tile([P, M], FP32, tag="swo", name="swo")
            nc.gpsimd.dma_start(
                t[:], wo_d[2 * p:2 * p + 2].rearrange("h d m -> (h d) m")
            )
            stg["wo"] = t
            return stg

        def load_rs_group(sg):
            tiles = [rs_pool.tile([P, M], FP32, tag="rs", name="rs") for _ in range(4)]
            for sci in range(4):
                q = nc.sync if sci % 2 == 0 else nc.gpsimd
                q.dma_start(
                    tiles[sci][:],
                    resid_d[(sg * 4 + sci) * P:(sg * 4 + sci + 1) * P, :],
                )
            return tiles

        wo_handles = []

        def convert_weights(stg, wr_out):
            """fp32 staging -> bf16 (DVE). Appends wq/wk/wv to wr_out, wo to
            wo_handles."""
            for name in ("wq", "wk", "wv"):
                wr = wr_pool.tile([P, MC, 2, D], BF16, tag=f"r{name}", name=f"r{name}")
                nc.vector.tensor_copy(wr[:], stg[name][:])
                wr_out.append(wr)
            wo_r = wo_pool.tile([P, M], BF16, tag="wo", name="wo_r")
            nc.vector.tensor_copy(wo_r[:], stg["wo"][:])
            wo_handles.append(wo_r)

        # ------------- lazy chain builders (alloc PSUM at first run) -------
        def proj_chain(wr_list, wi, T, sj):
            """T[:, sj*512:(sj+1)*512] = W^T @ residT[sj]: 2x(4 matmuls)+copy."""
            hold = {}

            def step(k):
                def f():
                    if "ps" not in hold:
                        hold["ps"] = ps_aux.tile([P, 512], FP32, tag="aux", name="aux")
                    ps = hold["ps"]
                    if k < 4:
                        wr = wr_list[wi]
                        for mi in range(2 * k, 2 * k + 2):
                            nc.tensor.matmul(
                                ps[:],
                                wr[:, mi].rearrange("pp h d -> pp (h d)"),
                                residT[sj][:, mi, :],
                                start=(mi == 0),
                                stop=(mi == MC - 1),
                            )
                    else:
                        nc.vector.tensor_copy(
                            T[:, sj * 512:(sj + 1) * 512], ps[:]
                        )
                return f

            return [step(0), step(1), step(2), step(3), step(4)]

        def vx_chain(VT, vx, kg):
            """vx[:, 4kg:4kg+4, :, 0:D] = transpose of 4 VT column blocks."""
            hold = {}

            def step(k):
                def f():
                    if "ps" not in hold:
                        hold["ps"] = ps_aux.tile([P, 512], FP32, tag="aux", name="aux")
                    tp = hold["ps"]
                    if k == 0:
                        for kbi in range(4):
                            kb = kg * 4 + kbi
                            nc.tensor.transpose(
                                tp[:, kbi * P:(kbi + 1) * P].bitcast(FP32R),
                                VT[:, kb * P:(kb + 1) * P],
                                ident_r[:],
                            )
                    else:
                        nc.vector.tensor_copy(
                            vx[:, kg * 4:(kg + 1) * 4, :, 0:D],
                            tp[:].rearrange(
                                "pp (kbi h d) -> pp kbi h d", kbi=4, h=2
                            ),
                        )
                return f

            return [step(0), step(1)]

        def out_chain(z_sbs, qb, mj, cp_eng=None):
            """out[qb*128:, mj*512:] = sum_p Z_p^T.T @ Wo_p."""
            hold = {}

            def step(k):
                def f():
                    if "ps" not in hold:
                        hold["ps"] = ps_aux.tile([P, 512], FP32, tag="aux", name="aux")
                    po = hold["ps"]
                    if k < 2:
                        for p4 in range(2 * k, 2 * k + 2):
                            nc.tensor.matmul(
                                po[:],
                                z_sbs[p4][:, qb * P:(qb + 1) * P],
                                wo_handles[p4][:, mj * 512:(mj + 1) * 512],
                                start=(p4 == 0),
                                stop=(p4 == NP - 1),
                            )
                    else:
                        ob = ob_pool.tile([P, 512], FP32, tag="o", name="ob")
                        if cp_eng is None:
                            nc.vector.tensor_copy(ob[:], po[:])
                        else:
                            nc.scalar.activation(ob[:], po[:], COPY)
                        nc.sync.dma_start(
                            out_d[qb * P:(qb + 1) * P,
                                  mj * 512:(mj + 1) * 512],
                            ob[:],
                        )
                return f

            return [step(0), step(1), step(2)]

        def enqueue_pair_chains(p, wr_list, QT, KT, VT, vx, direct_sg0=False):
            """Queue pair p's 12 projection chains + 4 vx chains, with a fence
            marker after each s-group."""
            for sg in range(4):
                thunks = (proj_chain(wr_list, 0, QT, sg)
                          + proj_chain(wr_list, 1, KT, sg)
                          + proj_chain(wr_list, 2, VT, sg)
                          + vx_chain(VT, vx, sg))
                if direct_sg0 and sg == 0:
                    for t in thunks:
                        t()
                    done_marks.add((p, 0))
                else:
                    fill.extend(thunks)
                    fill.append(("mark", (p, sg)))

        # ------------- pair 0 weights + resid transpose pipeline -----------
        # DMA issue order matters: the DMA engines are a serialized resource,
        # so order transfers by first-use time: sg0, wq, wk, wv, sg1, sg2,
        # wo, sg3.
        rs_first = load_rs_group(0)
        stg0 = {}
        for name, w_d in (("wq", wq_d), ("wk", wk_d), ("wv", wv_d)):
            t = wf_pool.tile([P, MC, 2, D], FP32, tag=f"s{name}", name=f"s{name}")
            for h in range(2):
                nc.gpsimd.dma_start(
                    t[:, :, h, :],
                    w_d[h].rearrange("(mc pp) d -> pp mc d", pp=P),
                )
            stg0[name] = t
        rs_g1 = load_rs_group(1)
        rs_g2 = load_rs_group(2)
        t = wf_pool.tile([P, M], FP32, tag="swo", name="swo")
        nc.gpsimd.dma_start(t[:], wo_d[0:2].rearrange("h d m -> (h d) m"))
        stg0["wo"] = t
        rs_g3 = load_rs_group(3)
        rs_pre = [None, rs_g1, rs_g2, rs_g3]
        wr0 = []
        convert_weights(stg0, wr0)

        QTs = [qk_pool.tile([P, S], BF16, tag="qt", name="QT0")]
        KTs = [qk_pool.tile([P, S], BF16, tag="kt", name="KT0")]
        VTs = [vt_pool.tile([P, S], FP32R, tag="vtt", name="VT0")]
        vxs = [vx_pool.tile([P, KB, 2, D + 1], BF16, tag="vx", name="vx0")]
        nc.vector.memset(vxs[0][:, :, :, D:D + 1], 1.0)

        rs_cur = rs_first
        for sg in range(4):
            rs_nxt = rs_pre[sg + 1] if sg < 3 else None
            for mi in range(MC):
                tp = ps_aux.tile([P, 512], FP32, tag="aux", name="tp")
                for sci in range(4):
                    nc.tensor.transpose(
                        tp[:, sci * P:(sci + 1) * P],
                        rs_cur[sci][:, mi * P:(mi + 1) * P],
                        ident_f[:],
                    )
                # alternate DVE/ACT for the fp32->bf16 convert-copy
                if mi % 2 == 0:
                    nc.vector.tensor_copy(residT[sg][:, mi, :], tp[:])
                else:
                    nc.scalar.activation(residT[sg][:, mi, :], tp[:], COPY)
            rs_cur = rs_nxt
            if sg == 0:
                enqueue_pair_chains(0, wr0, QTs[0], KTs[0], VTs[0], vxs[0],
                                    direct_sg0=True)

        # ------------- attention -------------
        z_sbs = []

        def normalize(zps, z_sb, qj):
            zsl = slice(qj * 512, (qj + 1) * 512)
            for h in range(2):
                rcp = rc_pool.tile([D + 1, 512], FP32, tag="rc", name="rcp")
                nc.vector.reciprocal(rcp[D:D + 1, :], zps[h][D:D + 1, :])
                Rs = rc_pool.tile([D, 512], FP32, tag="rsb", name="Rs")
                nc.sync.dma_start(
                    Rs[:], rcp[D:D + 1, None, :].to_broadcast((1, D, 512))
                )
                if h == 0:
                    nc.vector.tensor_mul(z_sb[0:D, zsl], zps[h][0:D, :], Rs[:])
                else:
                    ztmp = zt_pool.tile([D, 512], BF16, tag="zt", name="ztmp")
                    nc.vector.tensor_mul(ztmp[:], zps[h][0:D, :], Rs[:])
                    nc.sync.dma_start(z_sb[64:128, zsl], ztmp[:])

        def attention(p, QT, KT, vx, z_sb, qj_hooks, last_pair=False):
            pending = None
            for qj in range(QC):
                flush_until((p, qj))
                hook = qj_hooks.get(qj)
                if hook:
                    hook()
                nkb = 4 * qj + 4
                zps = None
                for kb in range(nkb):
                    m = kb - 4 * qj
                    c0 = 0 if m < 1 else P * m
                    st = ps_st.tile([P, 1024], FP32, tag="st", name="st")
                    if kb == 0:
                        zps = [
                            ps_z.tile([D + 1, 512], FP32, tag="z", name=f"z{h}")
                            for h in range(2)
                        ]
                    for h in range(2):
                        nc.tensor.matmul(
                            st[:, h * 512 + c0:(h + 1) * 512],
                            KT[h * D:(h + 1) * D, kb * P:(kb + 1) * P],
                            QT[h * D:(h + 1) * D,
                               qj * 512 + c0:(qj + 1) * 512],
                            start=True,
                            stop=True,
                        )
                    pt = pt_pool.tile([P, 1024], BF16, tag="pt", name="pt")
                    if c0 > 0:
                        st3 = st[:].rearrange("pp (h c) -> pp h c", h=2)
                        pt3 = pt[:].rearrange("pp (h c) -> pp h c", h=2)
                        nc.scalar.activation(
                            pt3[:, :, c0:512], st3[:, :, c0:512], EXP,
                            scale=0.125,
                        )
                    else:
                        nc.scalar.activation(pt[:], st[:], EXP, scale=0.125)
                    if m >= 0:
                        # zero the upper-triangle of the 128-col diag strip
                        for h in range(2):
                            nc.gpsimd.affine_select(
                                out=pt[:, h * 512 + c0:h * 512 + c0 + P],
                                in_=pt[:, h * 512 + c0:h * 512 + c0 + P],
                                compare_op=mybir.AluOpType.is_ge,
                                fill=0.0,
                                base=0,
                                pattern=[[1, P]],
                                channel_multiplier=-1,
                            )
                    # pair 3's out-chain filler must not run in the first
                    # blocks of a qj: it would stall on the normalize of the
                    # qj that just finished
                    if not (last_pair and qj > 0 and kb < 3):
                        pump(2 if last_pair else 1)
                    if pending is not None:
                        pending()

                    def mk_pv(zz, ptt, cc0, kkb, last):
                        def f():
                            for h in range(2):
                                nc.tensor.matmul(
                                    zz[h][:, cc0:512],
                                    vx[:, kkb, h, :],
                                    ptt[:, h * 512 + cc0:(h + 1) * 512],
                                    start=(kkb == 0),
                                    stop=last,
                                )
                        return f

                    pending = mk_pv(zps, pt, c0, kb, kb == nkb - 1)
                    if kb == nkb - 1:
                        # qj's last PV can't be delayed into the next block;
                        # extra filler covers the exp latency instead
                        pump(1)
                        pending()
                        pending = None
                        normalize(zps, z_sb, qj)
                        pump(2)

        for p in range(NP):
            z_sb = z_pool.tile([P, S], BF16, tag="z", name=f"zsb{p}")
            z_sbs.append(z_sb)

            if p < NP - 1:
                # queue next pair's weights + projections as filler
                pn = p + 1
                stg = stage_weights(pn)
                QTs.append(qk_pool.tile([P, S], BF16, tag="qt", name=f"QT{pn}"))
                KTs.append(qk_pool.tile([P, S], BF16, tag="kt", name=f"KT{pn}"))
                VTs.append(vt_pool.tile([P, S], FP32R, tag="vtt", name=f"VT{pn}"))
                vxs.append(vx_pool.tile([P, KB, 2, D + 1], BF16, tag="vx",
                                        name=f"vx{pn}"))
                wr_n = []

                def cvt(stg=stg, wr_n=wr_n, vxn=vxs[pn]):
                    convert_weights(stg, wr_n)
                    nc.vector.memset(vxn[:, :, :, D:D + 1], 1.0)

                fill.append(cvt)
                enqueue_pair_chains(pn, wr_n, QTs[pn], KTs[pn], VTs[pn],
                                    vxs[pn])

            qj_hooks = {}
            if p == NP - 1:
                # filler for the last pair: output projection of finished qj
                def mk_out_hook(qj_done):
                    def hk():
                        for qb in range(qj_done * 4, qj_done * 4 + 4):
                            for mj in range(2):
                                fill.extend(out_chain(z_sbs, qb, mj))
                    return hk

                for qj in range(1, QC):
                    qj_hooks[qj] = mk_out_hook(qj - 1)

            attention(p, QTs[p], KTs[p], vxs[p], z_sb, qj_hooks,
                      last_pair=(p == NP - 1))

        # ------------- tail: last q-chunk's output projection -------------
        flush()
        for i, (qb, mj) in enumerate([(qb, mj) for qb in range(12, KB)
                                      for mj in range(2)]):
            eng = None if i % 2 == 0 else nc.scalar
            for t in out_chain(z_sbs, qb, mj, cp_eng=eng):
                t()


_NC_CACHE = None


def _build_nc(split_waits=True):
    global _NC_CACHE
    if _NC_CACHE is not None and split_waits:
        return _NC_CACHE
    nc = bass.Bass("TRN2", target_bir_lowering=False, debug=False, num_devices=8)
    resid_d = nc.dram_tensor("resid", [S, M], FP32, kind="ExternalInput").ap()
    wq_d = nc.dram_tensor("wq", [H // 2, M, D], FP32, kind="ExternalInput").ap()
    wk_d = nc.dram_tensor("wk", [H // 2, M, D], FP32, kind="ExternalInput").ap()
    wv_d = nc.dram_tensor("wv", [H // 2, M, D], FP32, kind="ExternalInput").ap()
    wo_d = nc.dram_tensor("wo", [H // 2, D, M], FP32, kind="ExternalInput").ap()
    out_d = nc.dram_tensor("out", [S, M], FP32, kind="ExternalOutput").ap()
    with tile.TileContext(nc) as tc:
        _body(tc, nc, resid_d, wq_d, wk_d, wv_d, wo_d, out_d)
    if split_waits:
        _split_multiwait_instructions(nc)
        _NC_CACHE = nc
    return nc


def run(resid, w_q, w_k, w_v, w_o, **spmd_kwargs):
    """Build + run on 8 cores; returns (full output [4,2048,1024], BassKernelResults)."""
    resid = np.asarray(resid, dtype=np.float32)
    w_q = np.asarray(w_q, dtype=np.float32)
    w_k = np.asarray(w_k, dtype=np.float32)
    w_v = np.asarray(w_v, dtype=np.float32)
    w_o = np.asarray(w_o, dtype=np.float32)

    nc = _build_nc()
    in_maps = []
    for c in range(8):
        b, hh = c // 2, c % 2
        hs = slice(8 * hh, 8 * hh + 8)
        in_maps.append(
            {
                "resid": np.ascontiguousarray(resid[b]),
                "wq": np.ascontiguousarray(w_q[hs]),
                "wk": np.ascontiguousarray(w_k[hs]),
                "wv": np.ascontiguousarray(w_v[hs]),
                "wo": np.ascontiguousarray(w_o[hs]),
            }
        )
    res = run_bass_kernel_spmd(nc, in_maps, core_ids=list(range(8)), **spmd_kwargs)
    outs = [r["out"] for r in res.results]
    full = np.stack([outs[2 * b] + outs[2 * b + 1] for b in range(B)])
    return full.astype(np.float32), res


def kernel(resid, w_q, w_k, w_v, w_o):
    full, _ = run(resid, w_q, w_k, w_v, w_o)
    return full


# revision 12
# speedup vs baseline: 1.1213x; 1.0064x over previous
"""Multi-head causal attention (nn_Attention_29583734734990) on 8 Trainium2 cores.

Sharding: core c -> batch b = c//2, head half hh = c%2 (8 of 16 heads, as 4
head-pairs). Each core computes its partial output sum_{h in its 8 heads}
softmax(QK^T/sqrt(d), causal) V W_o[h] for its batch; the host adds the two
half-head partials per batch.

Schedule: software-pipelined by issue order. The attention inner loop's PE
work (scores + PV) is matched 1:1 by ACT exp rows, so with a naive order the
PE stalls on every block waiting for exp (and de-ramps its clock). Instead,
PE-only work -- the next pair's Q/K/V projection chains, V transposes, and
the output projection -- is kept in a FIFO of filler closures and pumped
between the score and PV matmuls of each block; PV is additionally delayed
one block so exp latency is fully hidden. Fence markers in the FIFO force
any chain a given q-chunk depends on to be issued before its first score.

Dtypes: bf16 PE datapath with fp32 PSUM accumulation (same per-row matmul
rate as fp32r, but no N>=256 full-rate restriction, so causally-dead columns
are trimmed to the exact 128-col diagonal strip). resid is transposed as
fp32r (1.5 c/row) straight from the fp32 DMA and converted to bf16 in the
PSUM->SBUF copy.

PSUM (8 banks): 2x score tiles [128,1024] (4) + 2x z accum [65,512] (2) +
2x aux [128,512] (2) shared by transposes/projections/output chains.
"""
from collections import deque
from contextlib import ExitStack

import numpy as np

import concourse.bass as bass
import concourse.mybir as mybir
import concourse.tile as tile
from concourse.bass_utils import run_bass_kernel_spmd
from concourse.masks import make_identity

FP32 = mybir.dt.float32
FP32R = mybir.dt.float32r
BF16 = mybir.dt.bfloat16
EXP = mybir.ActivationFunctionType.Exp
COPY = mybir.ActivationFunctionType.Copy

B, S, M, D, H = 4, 2048, 1024, 64, 16
P = 128
NP = 4          # head pairs per core
MC = M // P     # 8  m chunks
KB = S // P     # 16 k blocks
QC = S // 512   # 4  q chunks


def _split_multiwait_instructions(nc):
    """This walrus build rejects instructions carrying >1 sem-wait ("Too many
    sync wait commands"). Move extra waits onto single-wait NoOps inserted just
    before on the same engine queue (identical semantics)."""
    ctr = 0
    for fn in nc.m.functions:
        for bb in fn.blocks:
            new = []
            for inst in list(bb.instructions):
                si = inst.sync_info
                if si is not None and len(si.on_wait) > 1:
                    waits = list(si.on_wait)
                    for w in waits[:-1]:
                        ctr += 1
                        new.append(
                            mybir.InstNoOp(
                                name=f"I-splitw-{ctr}",
                                engine=inst.engine,
                                bass_nofuse=True,
                                sync_info=mybir.SyncInfo(on_wait=[w], on_update=[]),
                            )
                        )
                    inst.sync_info = mybir.SyncInfo(
                        on_wait=[waits[-1]], on_update=list(si.on_update)
                    )
                new.append(inst)
            bb.instructions = new
    return ctr


def _body(tc, nc, resid_d, wq_d, wk_d, wv_d, wo_d, out_d):
    with ExitStack() as ctx:
        const = ctx.enter_context(tc.tile_pool(name="const", bufs=1))
        ident_f = const.tile([P, P], FP32, name="ident_f")
        make_identity(nc, ident_f[:])
        ident_r = const.tile([P, P], FP32R, name="ident_r")
        nc.vector.tensor_copy(ident_r[:], ident_f[:])
        big = ctx.enter_context(tc.tile_pool(name="big", bufs=4))
        residT = [
            big.tile([P, MC, 512], BF16, tag="residT", name=f"residT{g}")
            for g in range(4)
        ]

        qk_pool = ctx.enter_context(tc.tile_pool(name="qk", bufs=2))
        vt_pool = ctx.enter_context(tc.tile_pool(name="vt", bufs=1))
        vx_pool = ctx.enter_context(tc.tile_pool(name="vx", bufs=2))
        z_pool = ctx.enter_context(tc.tile_pool(name="zsb", bufs=NP))
        wo_pool = ctx.enter_context(tc.tile_pool(name="wop", bufs=NP))
        wf_pool = ctx.enter_context(tc.tile_pool(name="wf", bufs=1))
        wr_pool = ctx.enter_context(tc.tile_pool(name="wr", bufs=2))
        rs_pool = ctx.enter_context(tc.tile_pool(name="rs", bufs=16))
        pt_pool = ctx.enter_context(tc.tile_pool(name="pt", bufs=3))
        rc_pool = ctx.enter_context(tc.tile_pool(name="rc", bufs=2))
        zt_pool = ctx.enter_context(tc.tile_pool(name="ztm", bufs=2))
        ob_pool = ctx.enter_context(tc.tile_pool(name="osb", bufs=3))

        ps_st = ctx.enter_context(tc.tile_pool(name="ps_st", bufs=2, space="PSUM"))
        ps_z = ctx.enter_context(tc.tile_pool(name="ps_z", bufs=2, space="PSUM"))
        ps_aux = ctx.enter_context(tc.tile_pool(name="ps_aux", bufs=2, space="PSUM"))

        # ------------- filler FIFO with fence markers -------------
        fill = deque()          # items: closures or ("mark", key)
        done_marks = set()

        def _run_one():
            item = fill.popleft()
            if isinstance(item, tuple):
                done_marks.add(item[1])
            else:
                item()

        def pump(n):
            for _ in range(n):
                if not fill:
                    return
                _run_one()

        def flush_until(key):
            while key not in done_marks and fill:
                _run_one()

        def flush():
            while fill:
                _run_one()

        # ------------- weights -------------
        def stage_weights(p):
            """Issue HBM loads for pair p's weights (SP + Pool queues)."""
            stg = {}
            for name, w_d, q in (
                ("wq", wq_d, nc.sync),
                ("wk", wk_d, nc.gpsimd),
                ("wv", wv_d, nc.sync),
            ):
                t = wf_pool.tile([P, MC, 2, D], FP32, tag=f"s{name}", name=f"s{name}")
                for h in range(2):
                    q.dma_start(
                        t[:, :, h, :],
                        w_d[2 * p + h].rearrange("(mc pp) d -> pp mc d", pp=P),
                    )
                stg[name] = t
            t = wf_pool.tile([P, M], FP32, tag="swo", name="swo")
            nc.gpsimd.dma_start(
                t[:], wo_d[2 * p:2 * p + 2].rearrange("h d m -> (h d) m")
            )
            stg["wo"] = t
            return stg

        def load_rs_group(sg):
            tiles = [rs_pool.tile([P, M], FP32, tag="rs", name="rs") for _ in range(4)]
            for sci in range(4):
                q = nc.sync if sci % 2 == 0 else nc.gpsimd
                q.dma_start(
                    tiles[sci][:],
                    resid_d[(sg * 4 + sci) * P:(sg * 4 + sci + 1) * P, :],
                )
            return tiles

        wo_handles = []

        def convert_weights(stg, wr_out):
            """fp32 staging -> bf16 (DVE). Appends wq/wk/wv to wr_out, wo to
            wo_handles."""
            for name in ("wq", "wk", "wv"):
                wr = wr_pool.tile([P, MC, 2, D], BF16, tag=f"r{name}", name=f"r{name}")
                nc.vector.tensor_copy(wr[:], stg[name][:])
                wr_out.append(wr)
            wo_r = wo_pool.tile([P, M], BF16, tag="wo", name="wo_r")
            nc.vector.tensor_copy(wo_r[:], stg["wo"][:])
            wo_handles.append(wo_r)

        # ------------- lazy chain builders (alloc PSUM at first run) -------
        def proj_chain(wr_list, wi, T, sj):
            """T[:, sj*512:(sj+1)*512] = W^T @ residT[sj]: 2x(4 matmuls)+copy."""
            hold = {}

            def step(k):
                def f():
                    if "ps" not in hold:
                        hold["ps"] = ps_aux.tile([P, 512], FP32, tag="aux", name="aux")
                    ps = hold["ps"]
                    if k < 4:
                        wr = wr_list[wi]
                        for mi in range(2 * k, 2 * k + 2):
                            nc.tensor.matmul(
                                ps[:],
                                wr[:, mi].rearrange("pp h d -> pp (h d)"),
                                residT[sj][:, mi, :],
                                start=(mi == 0),
                                stop=(mi == MC - 1),
                            )
                    else:
                        dst = T[:, sj * 512:(sj + 1) * 512]
                        if (wi + sj) % 2 == 0:
                            nc.vector.tensor_copy(dst, ps[:])
                        else:
                            nc.scalar.activation(dst, ps[:], COPY)
                return f

            return [step(0), step(1), step(2), step(3), step(4)]

        def vx_chain(VT, vx, kg):
            """vx[:, 4kg:4kg+4, :, 0:D] = transpose of 4 VT column blocks."""
            hold = {}

            def step(k):
                def f():
                    if "ps" not in hold:
                        hold["ps"] = ps_aux.tile([P, 512], FP32, tag="aux", name="aux")
                    tp = hold["ps"]
                    if k == 0:
                        for kbi in range(4):
                            kb = kg * 4 + kbi
                            nc.tensor.transpose(
                                tp[:, kbi * P:(kbi + 1) * P].bitcast(FP32R),
                                VT[:, kb * P:(kb + 1) * P],
                                ident_r[:],
                            )
                    else:
                        nc.vector.tensor_copy(
                            vx[:, kg * 4:(kg + 1) * 4, :, 0:D],
                            tp[:].rearrange(
                                "pp (kbi h d) -> pp kbi h d", kbi=4, h=2
                            ),
                        )
                return f

            return [step(0), step(1)]

        def out_chain(z_sbs, qb, mj):
            """out[qb*128:, mj*512:] = sum_p Z_p^T.T @ Wo_p."""
            hold = {}

            def step(k):
                def f():
                    if "ps" not in hold:
                        hold["ps"] = ps_aux.tile([P, 512], FP32, tag="aux", name="aux")
                    po = hold["ps"]
                    if k < 2:
                        for p4 in range(2 * k, 2 * k + 2):
                            nc.tensor.matmul(
                                po[:],
                                z_sbs[p4][:, qb * P:(qb + 1) * P],
                                wo_handles[p4][:, mj * 512:(mj + 1) * 512],
                                start=(p4 == 0),
                                stop=(p4 == NP - 1),
                            )
                    else:
                        ob = ob_pool.tile([P, 512], FP32, tag="o", name="ob")
                        if (qb + mj) % 2 == 0:
                            nc.vector.tensor_copy(ob[:], po[:])
                        else:
                            nc.scalar.activation(ob[:], po[:], COPY)
                        nc.sync.dma_start(
                            out_d[qb * P:(qb + 1) * P,
                                  mj * 512:(mj + 1) * 512],
                            ob[:],
                        )
                return f

            return [step(0), step(1), step(2)]

        def enqueue_pair_chains(p, wr_list, QT, KT, VT, vx, direct_sg0=False):
            """Queue pair p's 12 projection chains + 4 vx chains, with a fence
            marker after each s-group."""
            for sg in range(4):
                thunks = (proj_chain(wr_list, 0, QT, sg)
                          + proj_chain(wr_list, 1, KT, sg)
                          + proj_chain(wr_list, 2, VT, sg)
                          + vx_chain(VT, vx, sg))
                if direct_sg0 and sg == 0:
                    for t in thunks:
                        t()
                    done_marks.add((p, 0))
                else:
                    fill.extend(thunks)
                    fill.append(("mark", (p, sg)))

        # ------------- pair 0 weights + resid transpose pipeline -----------
        # DMA issue order matters: the DMA engines are a serialized resource,
        # so order transfers by first-use time: sg0, wq, wk, wv, sg1, sg2,
        # wo, sg3.
        rs_first = load_rs_group(0)
        stg0 = {}
        for name, w_d in (("wq", wq_d), ("wk", wk_d), ("wv", wv_d)):
            t = wf_pool.tile([P, MC, 2, D], FP32, tag=f"s{name}", name=f"s{name}")
            for h in range(2):
                nc.gpsimd.dma_start(
                    t[:, :, h, :],
                    w_d[h].rearrange("(mc pp) d -> pp mc d", pp=P),
                )
            stg0[name] = t
        rs_g1 = load_rs_group(1)
        rs_g2 = load_rs_group(2)
        t = wf_pool.tile([P, M], FP32, tag="swo", name="swo")
        nc.gpsimd.dma_start(t[:], wo_d[0:2].rearrange("h d m -> (h d) m"))
        stg0["wo"] = t
        rs_g3 = load_rs_group(3)
        rs_pre = [None, rs_g1, rs_g2, rs_g3]
        wr0 = []
        convert_weights(stg0, wr0)

        QTs = [qk_pool.tile([P, S], BF16, tag="qt", name="QT0")]
        KTs = [qk_pool.tile([P, S], BF16, tag="kt", name="KT0")]
        VTs = [vt_pool.tile([P, S], FP32R, tag="vtt", name="VT0")]
        vxs = [vx_pool.tile([P, KB, 2, D + 1], BF16, tag="vx", name="vx0")]
        nc.vector.memset(vxs[0][:, :, :, D:D + 1], 1.0)

        rs_cur = rs_first
        for sg in range(4):
            rs_nxt = rs_pre[sg + 1] if sg < 3 else None
            for mi in range(MC):
                tp = ps_aux.tile([P, 512], FP32, tag="aux", name="tp")
                for sci in range(4):
                    nc.tensor.transpose(
                        tp[:, sci * P:(sci + 1) * P],
                        rs_cur[sci][:, mi * P:(mi + 1) * P],
                        ident_f[:],
                    )
                # alternate DVE/ACT for the fp32->bf16 convert-copy
                if mi % 2 == 0:
                    nc.vector.tensor_copy(residT[sg][:, mi, :], tp[:])
                else:
                    nc.scalar.activation(residT[sg][:, mi, :], tp[:], COPY)
            rs_cur = rs_nxt
            if sg == 0:
                enqueue_pair_chains(0, wr0, QTs[0], KTs[0], VTs[0], vxs[0],
                                    direct_sg0=True)

        # ------------- attention -------------
        z_sbs = []

        def normalize(zps, z_sb, qj):
            zsl = slice(qj * 512, (qj + 1) * 512)
            for h in range(2):
                rcp = rc_pool.tile([D + 1, 512], FP32, tag="rc", name="rcp")
                nc.vector.reciprocal(rcp[D:D + 1, :], zps[h][D:D + 1, :])
                Rs = rc_pool.tile([D, 512], FP32, tag="rsb", name="Rs")
                nc.sync.dma_start(
                    Rs[:], rcp[D:D + 1, None, :].to_broadcast((1, D, 512))
                )
                if h == 0:
                    nc.vector.tensor_mul(z_sb[0:D, zsl], zps[h][0:D, :], Rs[:])
                else:
                    ztmp = zt_pool.tile([D, 512], BF16, tag="zt", name="ztmp")
                    nc.vector.tensor_mul(ztmp[:], zps[h][0:D, :], Rs[:])
                    nc.sync.dma_start(z_sb[64:128, zsl], ztmp[:])

        def attention(p, QT, KT, vx, z_sb, qj_hooks, last_pair=False):
            pending = None
            for qj in range(QC):
                flush_until((p, qj))
                hook = qj_hooks.get(qj)
                if hook:
                    hook()
                nkb = 4 * qj + 4
                zps = None
                for kb in range(nkb):
                    m = kb - 4 * qj
                    c0 = 0 if m < 1 else P * m
                    st = ps_st.tile([P, 1024], FP32, tag="st", name="st")
                    if kb == 0:
                        zps = [
                            ps_z.tile([D + 1, 512], FP32, tag="z", name=f"z{h}")
                            for h in range(2)
                        ]
                    for h in range(2):
                        nc.tensor.matmul(
                            st[:, h * 512 + c0:(h + 1) * 512],
                            KT[h * D:(h + 1) * D, kb * P:(kb + 1) * P],
                            QT[h * D:(h + 1) * D,
                               qj * 512 + c0:(qj + 1) * 512],
                            start=True,
                            stop=True,
                        )
                    pt = pt_pool.tile([P, 1024], BF16, tag="pt", name="pt")
                    if c0 > 0:
                        st3 = st[:].rearrange("pp (h c) -> pp h c", h=2)
                        pt3 = pt[:].rearrange("pp (h c) -> pp h c", h=2)
                        nc.scalar.activation(
                            pt3[:, :, c0:512], st3[:, :, c0:512], EXP,
                            scale=0.125,
                        )
                    else:
                        nc.scalar.activation(pt[:], st[:], EXP, scale=0.125)
                    if m >= 0:
                        # zero the upper-triangle of the 128-col diag strip
                        for h in range(2):
                            nc.gpsimd.affine_select(
                                out=pt[:, h * 512 + c0:h * 512 + c0 + P],
                                in_=pt[:, h * 512 + c0:h * 512 + c0 + P],
                                compare_op=mybir.AluOpType.is_ge,
                                fill=0.0,
                                base=0,
                                pattern=[[1, P]],
                                channel_multiplier=-1,
                            )
                    # pair 3's out-chain filler must not run in the first
                    # blocks of a qj: it would stall on the normalize of the
                    # qj that just finished
                    if not (last_pair and qj > 0 and kb < 3):
                        pump(2 if last_pair else 1)
                    if pending is not None:
                        pending()

                    def mk_pv(zz, ptt, cc0, kkb, last):
                        def f():
                            for h in range(2):
                                nc.tensor.matmul(
                                    zz[h][:, cc0:512],
                                    vx[:, kkb, h, :],
                                    ptt[:, h * 512 + cc0:(h + 1) * 512],
                                    start=(kkb == 0),
                                    stop=last,
                                )
                        return f

                    pending = mk_pv(zps, pt, c0, kb, kb == nkb - 1)
                    if kb == nkb - 1:
                        # qj's last PV can't be delayed into the next block;
                        # extra filler covers the exp latency instead
                        pump(1)
                        pending()
                        pending = None
                        normalize(zps, z_sb, qj)
                        pump(2)

        for p in range(NP):
            z_sb = z_pool.tile([P, S], BF16, tag="z", name=f"zsb{p}")
            z_sbs.append(z_sb)

            if p < NP - 1:
                # queue next pair's weights + projections as filler
                pn = p + 1
                stg = stage_weights(pn)
                QTs.append(qk_pool.tile([P, S], BF16, tag="qt", name=f"QT{pn}"))
                KTs.append(qk_pool.tile([P, S], BF16, tag="kt", name=f"KT{pn}"))
                VTs.append(vt_pool.tile([P, S], FP32R, tag="vtt", name=f"VT{pn}"))
                vxs.append(vx_pool.tile([P, KB, 2, D + 1], BF16, tag="vx",
                                        name=f"vx{pn}"))
                wr_n = []

                def cvt(stg=stg, wr_n=wr_n, vxn=vxs[pn]):
                    convert_weights(stg, wr_n)
                    nc.vector.memset(vxn[:, :, :, D:D + 1], 1.0)

                fill.append(cvt)
                enqueue_pair_chains(pn, wr_n, QTs[pn], KTs[pn], VTs[pn],
                                    vxs[pn])

            qj_hooks = {}
            if p == NP - 1:
                # filler for the last pair: output projection of finished qj
                def mk_out_hook(qj_done):
                    def hk():
                        for qb in range(qj_done * 4, qj_done * 4 + 4):
                            for mj in range(2):
                                fill.extend(out_chain(z_sbs, qb, mj))
                    return hk

                for qj in range(1, QC):
                    qj_hooks[qj] = mk_out_hook(qj - 1)

            attention(p, QTs[p], KTs[p], vxs[p], z_sb, qj_hooks,
                      last_pair=(p == NP - 1))

        # ------------- tail: last q-chunk's output projection -------------
        flush()
        for qb in range(12, KB):
            for mj in range(2):
                for t in out_chain(z_sbs, qb, mj):
                    t()


_NC_CACHE = None


def _build_nc(split_waits=True):
    global _NC_CACHE
    if _NC_CACHE is not None and split_waits:
        return _NC_CACHE
    nc = bass.Bass("TRN2", target_bir_lowering=False, debug=False, num_devices=8)
    resid_d = nc.dram_tensor("resid", [S, M], FP32, kind="ExternalInput").ap()
    wq_d = nc.dram_tensor("wq", [H // 2, M, D], FP32, kind="ExternalInput").ap()
    wk_d = nc.dram_tensor("wk", [H // 2, M, D], FP32, kind="ExternalInput").ap()
    wv_d = nc.dram_tensor("wv", [H // 2, M, D], FP32, kind="ExternalInput").ap()
    wo_d = nc.dram_tensor("wo", [H // 2, D, M], FP32, kind="ExternalInput").ap()
    out_d = nc.dram_tensor("out", [S, M], FP32, kind="ExternalOutput").ap()
    with tile.TileContext(nc) as tc:
        _body(tc, nc, resid_d, wq_d, wk_d, wv_d, wo_d, out_d)
    if split_waits:
        _split_multiwait_instructions(nc)
        _NC_CACHE = nc
    return nc


def run(resid, w_q, w_k, w_v, w_o, **spmd_kwargs):
    """Build + run on 8 cores; returns (full output [4,2048,1024], BassKernelResults)."""
    resid = np.asarray(resid, dtype=np.float32)
    w_q = np.asarray(w_q, dtype=np.float32)
    w_k = np.asarray(w_k, dtype=np.float32)
    w_v = np.asarray(w_v, dtype=np.float32)
    w_o = np.asarray(w_o, dtype=np.float32)

    nc = _build_nc()
    in_maps = []
    for c in range(8):
        b, hh = c // 2, c % 2
        hs = slice(8 * hh, 8 * hh + 8)
        in_maps.append(
            {
                "resid": np.ascontiguousarray(resid[b]),
                "wq": np.ascontiguousarray(w_q[hs]),
                "wk": np.ascontiguousarray(w_k[hs]),
                "wv": np.ascontiguousarray(w_v[hs]),
                "wo": np.ascontiguousarray(w_o[hs]),
            }
        )
    res = run_bass_kernel_spmd(nc, in_maps, core_ids=list(range(8)), **spmd_kwargs)
    outs = [r["out"] for r in res.results]
    full = np.stack([outs[2 * b] + outs[2 * b + 1] for b in range(B)])
    return full.astype(np.float32), res


def kernel(resid, w_q, w_k, w_v, w_o):
    full, _ = run(resid, w_q, w_k, w_v, w_o)
    return full
